# revision 5
# baseline (speedup 1.0000x reference)
"""Trainium2 Bass kernel for the gated-cell module:

    rt = sigmoid(xt @ Wa.T + ba); it = sigmoid(xt @ Wx.T + bx)
    at = exp(-(C*softplus(Lambda)) * rt)
    ht = at * ht_minus_1 + sqrt(1 - at^2) * (it * xt)

Sharding: data-parallel over the batch dim across 8 NeuronCores; weights
replicated.  Compute runs in a transposed layout ([D, B] with D on the
partition axis) so the per-feature scale/bias vectors (ba, bx,
-C*softplus(Lambda)) ride in the ACT engine's per-partition scale/bias
operands for free, and the xt operand is already K-major for the PE.

sqrt(1-at^2) is computed as exp(0.5*ln(1-at^2)) so the whole post-sigmoid
chain stays in the single `natural_log_exp_and_others` ACT table set
(the hardware sqrt table has a 65536-ULP error budget; ln/exp are tight).
"""

import sys

if "/opt/trn_rl_repo" not in sys.path:
    sys.path.insert(0, "/opt/trn_rl_repo")

import numpy as np

B, D = 16384, 1024
C = 8.0
NCORES = 8
BS = B // NCORES          # 2048 batch rows per core
PT = 128                  # partition tile
KT = D // PT              # 8 k-tiles (contraction)
JT = D // PT              # 8 j-tiles (output features)
CHUNKS = (1024, 1024)     # batch-chunk widths per core (sum == BS)

_CACHE = {}


def _build(dtype_tag="f32"):
    from contextlib import ExitStack

    import concourse.mybir as mybir
    import concourse.tile as tile
    from concourse import bacc

    f32 = mybir.dt.float32
    f32r = mybir.dt.float32r
    AF = mybir.ActivationFunctionType

    nc = bacc.Bacc("TRN2", target_bir_lowering=False, debug=False,
                   num_devices=NCORES)

    xtT = nc.dram_tensor("xtT", [D, BS], f32r, kind="ExternalInput").ap()
    htT = nc.dram_tensor("htT", [D, BS], f32, kind="ExternalInput").ap()
    waT = nc.dram_tensor("waT", [D, D], f32r, kind="ExternalInput").ap()
    wxT = nc.dram_tensor("wxT", [D, D], f32r, kind="ExternalInput").ap()
    biasA = nc.dram_tensor("biasA", [PT, JT], f32, kind="ExternalInput").ap()
    biasX = nc.dram_tensor("biasX", [PT, JT], f32, kind="ExternalInput").ap()
    negk = nc.dram_tensor("negk", [PT, JT], f32, kind="ExternalInput").ap()
    outT = nc.dram_tensor("outT", [D, BS], f32, kind="ExternalOutput").ap()

    with tile.TileContext(nc) as tc, ExitStack() as ctx:
        wpool = ctx.enter_context(tc.tile_pool(name="w", bufs=1))
        cpool = ctx.enter_context(tc.tile_pool(name="c", bufs=1))
        xpool = ctx.enter_context(tc.tile_pool(name="x", bufs=1))
        gpool = ctx.enter_context(tc.tile_pool(name="g", bufs=1))
        tpool = ctx.enter_context(tc.tile_pool(name="t", bufs=2))
        pzpool = ctx.enter_context(tc.tile_pool(name="pz", bufs=1, space="PSUM"))
        papool = ctx.enter_context(tc.tile_pool(name="pa", bufs=2, space="PSUM"))

        # Replicated weights, K-major: wa_sb[p, k, j] = Wa.T[k*128+p, j]
        wa_sb = [wpool.tile([PT, D], f32r, name=f"wa{k}", tag=f"wa{k}")
                 for k in range(KT)]
        wx_sb = [wpool.tile([PT, D], f32r, name=f"wx{k}", tag=f"wx{k}")
                 for k in range(KT)]
        for k in range(KT):
            nc.sync.dma_start(out=wa_sb[k], in_=waT[k * PT:(k + 1) * PT, :])
            nc.sync.dma_start(out=wx_sb[k], in_=wxT[k * PT:(k + 1) * PT, :])

        biasA_sb = cpool.tile([PT, JT], f32, tag="ba")
        biasX_sb = cpool.tile([PT, JT], f32, tag="bx")
        negk_sb = cpool.tile([PT, JT], f32, tag="nk")
        nc.sync.dma_start(out=biasA_sb, in_=biasA)
        nc.sync.dma_start(out=biasX_sb, in_=biasX)
        nc.sync.dma_start(out=negk_sb, in_=negk)

        coff = 0
        for ci, Q in enumerate(CHUNKS):
            bsl = slice(coff, coff + Q)
            coff += Q
            NH = Q // 512

            x_sb = [xpool.tile([PT, Q], f32r, name=f"xc{ci}k{k}", tag=f"x{k}")
                    for k in range(KT)]
            for k in range(KT):
                nc.sync.dma_start(out=x_sb[k], in_=xtT[k * PT:(k + 1) * PT, bsl])

            rt_g = gpool.tile([PT, JT, Q], f32, tag="rt")
            p_g = gpool.tile([PT, JT, Q], f32, tag="p")

            # ---- phase 1: GEMMs (fp32r) + sigmoids; p = it * xt ----
            for j in range(JT):
                jsl = slice(j * PT, (j + 1) * PT)
                za = pzpool.tile([PT, Q], f32, tag="za")
                zx = pzpool.tile([PT, Q], f32, tag="zx")
                for k in range(KT):
                    lhs_a = wa_sb[k][:, jsl]
                    rhs = x_sb[k]
                    for h in range(NH):
                        nsl = slice(h * 512, (h + 1) * 512)
                        nc.tensor.matmul(za[:, nsl], lhs_a, rhs[:, nsl],
                                         start=(k == 0), stop=(k == KT - 1))
                for k in range(KT):
                    lhs_x = wx_sb[k][:, jsl]
                    rhs = x_sb[k]
                    for h in range(NH):
                        nsl = slice(h * 512, (h + 1) * 512)
                        nc.tensor.matmul(zx[:, nsl], lhs_x, rhs[:, nsl],
                                         start=(k == 0), stop=(k == KT - 1))
                nc.scalar.activation(out=rt_g[:, j, :], in_=za, func=AF.Sigmoid,
                                     bias=biasA_sb[:, j:j + 1], scale=1.0)
                nc.scalar.activation(out=p_g[:, j, :], in_=zx, func=AF.Sigmoid,
                                     bias=biasX_sb[:, j:j + 1], scale=1.0)
                nc.vector.tensor_mul(p_g[:, j, :], p_g[:, j, :], x_sb[j].bitcast(f32))

            # ---- phase 2: at, sqrt(1-at^2) via ln/exp, combine, store ----
            for j in range(JT):
                jsl = slice(j * PT, (j + 1) * PT)
                h_t = tpool.tile([PT, Q], f32, tag="h")
                nc.sync.dma_start(out=h_t, in_=htT[jsl, bsl])

                at_t = tpool.tile([PT, Q], f32, tag="at", name=f"at{ci}_{j}")
                nc.scalar.activation(out=at_t, in_=rt_g[:, j, :], func=AF.Exp,
                                     scale=negk_sb[:, j:j + 1])
                a2 = tpool.tile([PT, Q], f32, tag="a2")
                nc.vector.tensor_mul(a2, at_t, at_t)
                m1 = tpool.tile([PT, Q], f32, tag="m1")
                nc.vector.tensor_mul(m1, at_t, h_t)
                # a2 <- ln(1 - a2), then a2 <- exp(0.5*ln(...)) = sqrt(1-at^2)
                nc.scalar.activation(out=a2, in_=a2, func=AF.Ln,
                                     bias=1.0, scale=-1.0)
                nc.scalar.activation(out=a2, in_=a2, func=AF.Exp, scale=0.5)
                m3 = tpool.tile([PT, Q], f32, tag="m3", bufs=1)
                nc.vector.tensor_mul(m3, a2, p_g[:, j, :])
                o = tpool.tile([PT, Q], f32, tag="o")
                nc.vector.tensor_add(o, m1, m3)
                nc.sync.dma_start(out=outT[jsl, bsl], in_=o)

    nc.compile()
    return nc


def _np_softplus(x):
    return np.logaddexp(0.0, x)


def _fold(vec):
    # [D] feature vector -> [128, JT] tile where column j holds features
    # j*128 .. j*128+127 (per-partition scalars for j-tile j).
    return np.ascontiguousarray(vec.reshape(JT, PT).T)


def kernel(xt, ht_minus_1, Wa, Wx, ba, bx, Lambda):
    from concourse.bass_utils import run_bass_kernel_spmd

    if "nc" not in _CACHE:
        _CACHE["nc"] = _build()
    nc = _CACHE["nc"]

    xt = np.asarray(xt, dtype=np.float32)
    ht = np.asarray(ht_minus_1, dtype=np.float32)
    Wa = np.asarray(Wa, dtype=np.float32)
    Wx = np.asarray(Wx, dtype=np.float32)
    ba = np.asarray(ba, dtype=np.float32).reshape(-1)
    bx = np.asarray(bx, dtype=np.float32).reshape(-1)
    Lam = np.asarray(Lambda, dtype=np.float32).reshape(-1)

    negk_vec = (-C * _np_softplus(Lam.astype(np.float64))).astype(np.float32)

    xtT = np.ascontiguousarray(xt.T)
    htT = np.ascontiguousarray(ht.T)
    waT = np.ascontiguousarray(Wa.T)
    wxT = np.ascontiguousarray(Wx.T)
    biasA = _fold(ba)
    biasX = _fold(bx)
    negk = _fold(negk_vec)

    in_maps = []
    for c in range(NCORES):
        sl = slice(c * BS, (c + 1) * BS)
        in_maps.append({
            "xtT": np.ascontiguousarray(xtT[:, sl]),
            "htT": np.ascontiguousarray(htT[:, sl]),
            "waT": waT,
            "wxT": wxT,
            "biasA": biasA,
            "biasX": biasX,
            "negk": negk,
        })

    res = run_bass_kernel_spmd(nc, in_maps, list(range(NCORES)))
    outT = np.concatenate([res.results[c]["outT"] for c in range(NCORES)],
                          axis=1)
    return np.ascontiguousarray(outT.T)
